# revision 8
# baseline (speedup 1.0000x reference)
"""SAGEConv(aggr='max') Trainium2 kernel, sharded over 8 NeuronCores.

Problem:  out_i = W_l @ max_{j in N(i)} x_j + b_l + W_r @ x_i
          X [50000,128] f32, edge_index [2,800000] int64, out [50000,1] f32.

Strategy (dst-sharded, 8 cores), v3.1 — bf16 transposed slot-major stream:
  - Each core owns 6250 destination nodes; edges partitioned by dst.  Host
    sorts each core's nodes by in-degree (descending).  Slot k holds the
    (k+1)-th edge of every node that has one; because nodes are degree-
    sorted, slot k covers exactly the first m_k nodes (m_k = max over
    cores of #nodes with deg > k, even-rounded; shared so one SPMD
    program serves all cores).  Pad entries duplicate the node's first
    edge (max is idempotent); degree-0 nodes get zero rows (PyG fill).
  - TRANSPOSED layout: feature dims on the 128 partitions, nodes on the
    free axis.  Slot-region k is [128, m_k] bf16 in DRAM.
  - Device dataflow:
      DMA   : acc[:, :6272] is initialized from slot 0 (split in two so
              the chain can start early); slots k>=1 stream through a
              ring of SBUF buffers.  Chain order = a few small tail slots
              first (only need the acc head), big slots, then the very
              smallest slots last so the post-stream tail is tiny.
      DVE   : ONE bf16 tensor_tensor max per slot over acc[:, :m_k]
              (2x perf mode) — the only vector work.
      PE    : per node-chunk (128-wide chunk for the highest-degree tile,
              512 elsewhere), W_l.agg + W_r.own via two accumulating
              matmuls into PSUM, issued as soon as the chunk's slots are
              folded (low-degree chunks finish mid-stream).
      ACT   : drains each PSUM chunk to the [1, 6272] f32 output row
              (+b_l); output DMA is split so only the final 128 nodes
              remain after the last chain op.
  - bf16 halves HBM traffic vs f32 (~27.4MB/core); the kernel runs at the
    measured ~26GiB/s-per-SDMA-engine ceiling (~418GB/s/core).
  - Host unpermutes the per-core output rows back to global node order.
"""

import numpy as np
import ml_dtypes

N_NODES = 50000
N_EDGES = 800000
D_IN = 128
N_CORES = 8
NPC = N_NODES // N_CORES  # 6250 nodes per core
P = 128
NT = (NPC + P - 1) // P  # 49 tiles of 128 nodes
NODES_PAD = NT * P  # 6272

F32 = np.float32
BF16 = ml_dtypes.bfloat16

NRING = 8  # ring depth for streaming slot-group buffers
GROUP_TARGET = 6272  # elems/partition per DMA group (= 1.57MB at bf16)
TAIL_W = 768  # slots narrower than this are "tail" slots
CHUNK = 512  # nodes per PE/PSUM chunk (first chunk is 128)
NPSUM = 4  # psum ring depth


# ---------------------------------------------------------------- host side
def _plan(m_k):
    """Chain order, DMA groups, PE chunks from slot widths m_k.

    Returns dict with:
      chain   : [(k, pos)] slot fold order
      groups  : [(k_lo, k_hi)] DMA groups (contiguous k ranges, chain order)
      gq_last : per group, chain pos of its last fold (ring credit)
      ma      : acc head size needed by the early tail folds
      chunks  : [(c0, c1, need)] PE chunks in execution order
    """
    Kmax = len(m_k)
    ks = list(range(1, Kmax))
    big = [k for k in ks if m_k[k] > TAIL_W]
    tail = [k for k in ks if m_k[k] <= TAIL_W]
    t1 = tail[: len(tail) // 2]
    t2 = tail[len(tail) // 2 :]
    if not t1:  # degenerate: few slots
        t1, t2 = tail, []

    chain_ks = t1 + big + t2
    pos = {k: i for i, k in enumerate(chain_ks)}

    def contiguous_groups(kl, cap):
        out = []
        i = 0
        while i < len(kl):
            j = i
            w = 0
            while (
                j < len(kl)
                and kl[j] == kl[i] + (j - i)
                and (w == 0 or w + m_k[kl[j]] <= cap)
            ):
                w += m_k[kl[j]]
                j += 1
            out.append((kl[i], kl[j - 1] + 1))
            i = j
        return out

    groups = (
        contiguous_groups(t1, GROUP_TARGET)
        + contiguous_groups(big, GROUP_TARGET)
        + contiguous_groups(t2, GROUP_TARGET)
    )
    gq_last = [max(pos[k] for k in range(lo, hi)) for lo, hi in groups]
    ma = max((m_k[k] for k in t1), default=2)

    c0s = [0] + [P + CHUNK * i for i in range((NODES_PAD - P) // CHUNK)]
    chunks = []
    for c0 in c0s:
        c1 = min(c0 + (P if c0 == 0 else CHUNK), NODES_PAD)
        need = 0
        for k in chain_ks:
            if m_k[k] > c0:
                need = max(need, pos[k] + 1)
        chunks.append((c0, c1, need))
    chunks.sort(key=lambda t: (t[2], -t[0]))
    return {
        "chain": [(k, pos[k]) for k in chain_ks],
        "groups": groups,
        "gq_last": gq_last,
        "ma": ma,
        "chunks": chunks,
    }


def _widths(K_tiles_unused, deg_sorted_all):
    """m_k widths shared across cores (elementwise max, even-rounded)."""
    Kmax = int(max(int(d[0]) for d in deg_sorted_all))
    Kmax = max(Kmax, 1)
    m_k = [NODES_PAD]
    for k in range(1, Kmax):
        m = max(int((d > k).sum()) for d in deg_sorted_all)
        m = max(m, 2)
        m_k.append(m + (m & 1))
    return m_k


def _preprocess(X, W_l, b_l, W_r, edge_index):
    X = np.asarray(X, dtype=F32)
    W_l = np.asarray(W_l, dtype=F32).reshape(-1)
    W_r = np.asarray(W_r, dtype=F32).reshape(-1)
    b_l = float(np.asarray(b_l).reshape(-1)[0])

    src = np.asarray(edge_index[0], dtype=np.int64)
    dst = np.asarray(edge_index[1], dtype=np.int64)
    core = dst // NPC

    # X^T in bf16 with a trailing all-zero column: index N_NODES = "empty".
    xzT = np.zeros((D_IN, N_NODES + 1), dtype=BF16)
    xzT[:, :N_NODES] = X.T.astype(BF16)

    per_core = []
    deg_sorted_all = []
    for c in range(N_CORES):
        sel = core == c
        s = src[sel]
        d = dst[sel] - c * NPC
        deg = np.bincount(d, minlength=NPC)
        order = np.argsort(-deg, kind="stable")  # local ids, degree desc
        deg_sorted = np.zeros(NODES_PAD, dtype=np.int64)
        deg_sorted[:NPC] = deg[order]
        deg_sorted_all.append(deg_sorted)

        eorder = np.argsort(d, kind="stable")
        d_s = d[eorder]
        s_s = s[eorder]
        start = np.zeros(NPC + 1, dtype=np.int64)
        np.cumsum(deg, out=start[1:])
        rank = np.arange(len(d_s), dtype=np.int64) - start[d_s]
        ipos = np.empty(NPC, dtype=np.int64)  # local id -> sorted position
        ipos[order] = np.arange(NPC)
        per_core.append((order, deg_sorted, ipos[d_s], rank, s_s))

    m_k = _widths(None, deg_sorted_all)
    Kmax = len(m_k)
    W_slots = sum(m_k)

    in_maps = []
    orders = []
    for c in range(N_CORES):
        order, deg_sorted, pos_e, rank_e, s_s = per_core[c]
        table = np.full((NODES_PAD, Kmax), N_NODES, dtype=np.int64)
        table[pos_e, rank_e] = s_s
        dup = table[:, 0]  # first edge src, or zero-col for degree-0 nodes
        cols = np.arange(Kmax, dtype=np.int64)[None, :]
        table = np.where(cols < deg_sorted[:, None], table, dup[:, None])

        # slot-major transposed neighbor table [128 dims, sum_k m_k]
        xg = np.empty((P, W_slots), dtype=BF16)
        off = 0
        for k in range(Kmax):
            m = m_k[k]
            xg[:, off : off + m] = xzT[:, table[:m, k]]
            off += m

        # own features transposed [128 dims, NODES_PAD]
        xo = np.zeros((P, NODES_PAD), dtype=BF16)
        xo[:, :NPC] = xzT[:, c * NPC + order]

        w2 = np.zeros((P, 2), dtype=BF16)
        w2[:, 0] = W_l.astype(BF16)
        w2[:, 1] = W_r.astype(BF16)

        in_maps.append({"xg": xg, "xo": xo, "w2": w2})
        orders.append(order)

    return in_maps, orders, m_k, b_l


def _assemble(results, orders):
    out = np.empty((N_NODES, 1), dtype=F32)
    for c in range(N_CORES):
        oc = np.asarray(results[c]["out"]).reshape(-1)  # [NODES_PAD]
        out[c * NPC + orders[c], 0] = oc[:NPC]
    return out


# -------------------------------------------------------------- device side
def _build_program(m_k, b_l):
    import concourse.bass as bass
    import concourse.mybir as mybir
    from contextlib import ExitStack

    f32 = mybir.dt.float32
    bf16 = mybir.dt.bfloat16
    plan = _plan(m_k)
    chain = plan["chain"]
    groups = plan["groups"]
    gq_last = plan["gq_last"]
    ma = plan["ma"]
    chunks = plan["chunks"]
    Kmax = len(m_k)
    W_slots = sum(m_k)
    W_acc = NODES_PAD
    offs = np.zeros(Kmax + 1, dtype=np.int64)
    np.cumsum(np.asarray(m_k), out=offs[1:])
    pos = dict(chain)
    NG = len(groups)
    n_chain = len(chain)

    nc = bass.Bass()
    xg = nc.declare_dram_parameter("xg", [P, W_slots], bf16, isOutput=False)
    xo = nc.declare_dram_parameter("xo", [P, NODES_PAD], bf16, isOutput=False)
    w2_d = nc.declare_dram_parameter("w2", [P, 2], bf16, isOutput=False)
    out = nc.declare_dram_parameter("out", [1, NODES_PAD], f32, isOutput=True)

    with ExitStack() as ctx:
        block = ctx.enter_context(nc.Block())
        s_aa = ctx.enter_context(nc.semaphore("s_aa"))  # slot0 head landed
        s_ab = ctx.enter_context(nc.semaphore("s_ab"))  # slot0 rest landed
        s_w = ctx.enter_context(nc.semaphore("s_w"))  # w2 + xo landed
        s_v = ctx.enter_context(nc.semaphore("s_v"))  # chain folds completed
        s_p = ctx.enter_context(nc.semaphore("s_p"))  # PE chunk pairs done
        s_ad = ctx.enter_context(nc.semaphore("s_ad"))  # ACT chunks drained
        s_out = ctx.enter_context(nc.semaphore("s_out"))
        sg = [ctx.enter_context(nc.semaphore(f"sg{b}")) for b in range(NRING)]

        w_t = ctx.enter_context(nc.sbuf_tensor("w_t", [P, 2], bf16))
        acc = ctx.enter_context(nc.sbuf_tensor("acc", [P, W_acc], bf16))
        xo_t = ctx.enter_context(nc.sbuf_tensor("xo_t", [P, NODES_PAD], bf16))
        orow = ctx.enter_context(nc.sbuf_tensor("orow", [1, NODES_PAD], f32))
        gq = [
            ctx.enter_context(nc.sbuf_tensor(f"gq{b}", [P, GROUP_TARGET], bf16))
            for b in range(NRING)
        ]
        ps = [
            ctx.enter_context(nc.psum_tensor(f"ps{i}", [1, CHUNK], f32))
            for i in range(NPSUM)
        ]

        @block.sync
        def _(sync):
            # acc head first: unblocks the early tail folds
            sync.dma_start(out=acc[:, :ma], in_=xg[:, :ma]).then_inc(s_aa, 16)
            first = True
            for g, (lo, hi) in enumerate(groups):
                b = g % NRING
                if g >= NRING:
                    sync.wait_ge(s_v, gq_last[g - NRING] + 1)
                width = int(offs[hi] - offs[lo])
                sync.dma_start(
                    out=gq[b][:, :width],
                    in_=xg[:, int(offs[lo]) : int(offs[hi])],
                ).then_inc(sg[b], 16)
                if first:
                    first = False
                    sync.dma_start(
                        out=acc[:, ma:], in_=xg[:, ma : int(offs[1])]
                    ).then_inc(s_ab, 16)
                    sync.dma_start(out=w_t[:], in_=w2_d[:]).then_inc(s_w, 16)
                    sync.dma_start(out=xo_t[:], in_=xo[:]).then_inc(s_w, 16)
            if first:  # no streamed slots at all (max degree 1)
                sync.dma_start(
                    out=acc[:, ma:], in_=xg[:, ma : int(offs[1])]
                ).then_inc(s_ab, 16)
                sync.dma_start(out=w_t[:], in_=w2_d[:]).then_inc(s_w, 16)
                sync.dma_start(out=xo_t[:], in_=xo[:]).then_inc(s_w, 16)
            # all but the highest-degree 128 nodes
            sync.wait_ge(s_ad, len(chunks) - 1)
            sync.dma_start(out=out[:, P:], in_=orow[:, P:]).then_inc(s_out, 16)
            sync.wait_ge(s_ad, len(chunks))
            sync.dma_start(out=out[:, :P], in_=orow[:, :P]).then_inc(s_out, 16)
            sync.wait_ge(s_out, 32)

        @block.vector
        def _(v):
            v.wait_ge(s_aa, 16)
            seen_big = False
            use = [0] * NRING
            for g, (lo, hi) in enumerate(groups):
                b = g % NRING
                use[b] += 1
                v.wait_ge(sg[b], 16 * use[b])
                for k in range(lo, hi):
                    m = m_k[k]
                    if not seen_big and m > ma:
                        v.wait_ge(s_ab, 16)
                        seen_big = True
                    goff = int(offs[k] - offs[lo])
                    v.tensor_tensor(
                        out=acc[:, :m],
                        in0=acc[:, :m],
                        in1=gq[b][:, goff : goff + m],
                        op=mybir.AluOpType.max,
                    ).then_inc(s_v, 1)

        @block.tensor
        def _(te):
            te.wait_ge(s_w, 32)
            for i, (c0, c1, need) in enumerate(chunks):
                wdt = c1 - c0
                if need > 0:
                    te.wait_ge(s_v, need)
                if i >= NPSUM:
                    te.wait_ge(s_ad, i - NPSUM + 1)
                pb = ps[i % NPSUM]
                te.matmul(
                    pb[:, :wdt],
                    w_t[:, 0:1],
                    acc[:, c0:c1],
                    start=True,
                    stop=False,
                )
                te.matmul(
                    pb[:, :wdt],
                    w_t[:, 1:2],
                    xo_t[:, c0:c1],
                    start=False,
                    stop=True,
                ).then_inc(s_p, 1)

        @block.scalar
        def _(a):
            for i, (c0, c1, need) in enumerate(chunks):
                wdt = c1 - c0
                a.wait_ge(s_p, i + 1)
                if b_l == 0.0:
                    ins = a.activation(
                        out=orow[:, c0:c1],
                        in_=ps[i % NPSUM][:, :wdt],
                        func=mybir.ActivationFunctionType.Copy,
                    )
                else:
                    ins = a.activation(
                        out=orow[:, c0:c1],
                        in_=ps[i % NPSUM][:, :wdt],
                        func=mybir.ActivationFunctionType.Identity,
                        bias=float(b_l),
                    )
                ins.then_inc(s_ad, 1)

    return nc


# ---------------------------------------------------------------- entry
def _run(inputs, trace=False, trace_cores=None):
    from concourse.bass_utils import run_bass_kernel_spmd

    in_maps, orders, m_k, b_l = _preprocess(**inputs)
    nc = _build_program(m_k, b_l)
    res = run_bass_kernel_spmd(
        nc,
        in_maps,
        core_ids=list(range(N_CORES)),
        trace=trace,
        trace_cores=trace_cores,
    )
    return _assemble(res.results, orders), res


def kernel(**inputs):
    out, _ = _run(inputs)
    return out


# revision 13
# speedup vs baseline: 1.0135x; 1.0135x over previous
"""SAGEConv(aggr='max') Trainium2 kernel, sharded over 8 NeuronCores.

Problem:  out_i = W_l @ max_{j in N(i)} x_j + b_l + W_r @ x_i
          X [50000,128] f32, edge_index [2,800000] int64, out [50000,1] f32.

Strategy (dst-sharded, 8 cores), v3.1 — bf16 transposed slot-major stream:
  - Each core owns 6250 destination nodes; edges partitioned by dst.  Host
    sorts each core's nodes by in-degree (descending).  Slot k holds the
    (k+1)-th edge of every node that has one; because nodes are degree-
    sorted, slot k covers exactly the first m_k nodes (m_k = max over
    cores of #nodes with deg > k, even-rounded; shared so one SPMD
    program serves all cores).  Pad entries duplicate the node's first
    edge (max is idempotent); degree-0 nodes get zero rows (PyG fill).
  - TRANSPOSED layout: feature dims on the 128 partitions, nodes on the
    free axis.  Slot-region k is [128, m_k] bf16 in DRAM.
  - Device dataflow:
      DMA   : acc[:, :6272] is initialized from slot 0 (split in two so
              the chain can start early); slots k>=1 stream through a
              ring of SBUF buffers.  Chain order = a few small tail slots
              first (only need the acc head), big slots, then the very
              smallest slots last so the post-stream tail is tiny.
      DVE   : ONE bf16 tensor_tensor max per slot over acc[:, :m_k]
              (2x perf mode) — the only vector work.
      PE    : per node-chunk (128-wide chunk for the highest-degree tile,
              512 elsewhere), W_l.agg + W_r.own via two accumulating
              matmuls into PSUM, issued as soon as the chunk's slots are
              folded (low-degree chunks finish mid-stream).
      ACT   : drains each PSUM chunk to the [1, 6272] f32 output row
              (+b_l); output DMA is split so only the final 128 nodes
              remain after the last chain op.
  - bf16 halves HBM traffic vs f32 (~27.4MB/core); the kernel runs at the
    measured ~26GiB/s-per-SDMA-engine ceiling (~418GB/s/core).
  - Host unpermutes the per-core output rows back to global node order.
"""

import numpy as np
import ml_dtypes

N_NODES = 50000
N_EDGES = 800000
D_IN = 128
N_CORES = 8
NPC = N_NODES // N_CORES  # 6250 nodes per core
P = 128
NT = (NPC + P - 1) // P  # 49 tiles of 128 nodes
NODES_PAD = NT * P  # 6272

F32 = np.float32
BF16 = ml_dtypes.bfloat16

NRING = 8  # ring depth for streaming slot-group buffers
GROUP_TARGET = 6272  # elems/partition per DMA group (= 1.57MB at bf16)
TAIL_W = 768  # slots narrower than this are "tail" slots
CHUNK = 512  # nodes per PE/PSUM chunk (first chunk is 128)
NPSUM = 4  # psum ring depth


# ---------------------------------------------------------------- host side
def _plan(m_k):
    """DMA groups and PE chunks from slot widths m_k (slots fold in
    ascending k order; m_k is non-increasing so the stream naturally ends
    with the narrow slots).

    Returns dict with:
      groups  : [(k_lo, k_hi)] DMA groups (contiguous ascending k)
      gq_last : per group, index of its last fold (= k_hi-1; ring credit)
      chunks  : [(c0, c1, need)] PE chunks in execution order; `need` is
                the number of folds that must complete first
    """
    Kmax = len(m_k)
    groups = []
    k = 1
    while k < Kmax:
        # keep the last groups small so the post-stream fold tail is tiny
        cap = GROUP_TARGET if m_k[k] > TAIL_W else TAIL_W
        lo = k
        w = 0
        while k < Kmax and (w == 0 or w + m_k[k] <= cap):
            w += m_k[k]
            k += 1
        groups.append((lo, k))
    gq_last = [hi - 1 for lo, hi in groups]

    c0s = [0] + [P + CHUNK * i for i in range((NODES_PAD - P) // CHUNK)]
    chunks = []
    for c0 in c0s:
        c1 = min(c0 + (P if c0 == 0 else CHUNK), NODES_PAD)
        need = sum(1 for k in range(1, Kmax) if m_k[k] > c0)
        chunks.append((c0, c1, need))
    chunks.sort(key=lambda t: (t[2], -t[0]))
    return {"groups": groups, "gq_last": gq_last, "chunks": chunks}


def _widths(deg_sorted_all):
    """m_k widths shared across cores (elementwise max, 32-rounded)."""
    Kmax = int(max(int(d[0]) for d in deg_sorted_all))
    Kmax = max(Kmax, 1)
    m_k = [NODES_PAD]
    for k in range(1, Kmax):
        m = max(int((d > k).sum()) for d in deg_sorted_all)
        m = min(-(-max(m, 32) // 32) * 32, NODES_PAD)
        m_k.append(m)
    return m_k


def _preprocess(X, W_l, b_l, W_r, edge_index):
    X = np.asarray(X, dtype=F32)
    W_l = np.asarray(W_l, dtype=F32).reshape(-1)
    W_r = np.asarray(W_r, dtype=F32).reshape(-1)
    b_l = float(np.asarray(b_l).reshape(-1)[0])

    src = np.asarray(edge_index[0], dtype=np.int64)
    dst = np.asarray(edge_index[1], dtype=np.int64)
    core = dst // NPC

    # X^T in bf16 with a trailing all-zero column: index N_NODES = "empty".
    xzT = np.zeros((D_IN, N_NODES + 1), dtype=BF16)
    xzT[:, :N_NODES] = X.T.astype(BF16)

    per_core = []
    deg_sorted_all = []
    for c in range(N_CORES):
        sel = core == c
        s = src[sel]
        d = dst[sel] - c * NPC
        deg = np.bincount(d, minlength=NPC)
        order = np.argsort(-deg, kind="stable")  # local ids, degree desc
        deg_sorted = np.zeros(NODES_PAD, dtype=np.int64)
        deg_sorted[:NPC] = deg[order]
        deg_sorted_all.append(deg_sorted)

        eorder = np.argsort(d, kind="stable")
        d_s = d[eorder]
        s_s = s[eorder]
        start = np.zeros(NPC + 1, dtype=np.int64)
        np.cumsum(deg, out=start[1:])
        rank = np.arange(len(d_s), dtype=np.int64) - start[d_s]
        ipos = np.empty(NPC, dtype=np.int64)  # local id -> sorted position
        ipos[order] = np.arange(NPC)
        per_core.append((order, deg_sorted, ipos[d_s], rank, s_s))

    m_k = _widths(deg_sorted_all)
    Kmax = len(m_k)
    W_slots = sum(m_k)

    in_maps = []
    orders = []
    for c in range(N_CORES):
        order, deg_sorted, pos_e, rank_e, s_s = per_core[c]
        table = np.full((NODES_PAD, Kmax), N_NODES, dtype=np.int64)
        table[pos_e, rank_e] = s_s
        dup = table[:, 0]  # first edge src, or zero-col for degree-0 nodes
        cols = np.arange(Kmax, dtype=np.int64)[None, :]
        table = np.where(cols < deg_sorted[:, None], table, dup[:, None])

        # slot-major transposed neighbor table [128 dims, sum_k m_k]
        xg = np.empty((P, W_slots), dtype=BF16)
        off = 0
        for k in range(Kmax):
            m = m_k[k]
            xg[:, off : off + m] = xzT[:, table[:m, k]]
            off += m

        # own features transposed [128 dims, NODES_PAD]
        xo = np.zeros((P, NODES_PAD), dtype=BF16)
        xo[:, :NPC] = xzT[:, c * NPC + order]

        w2 = np.zeros((P, 2), dtype=BF16)
        w2[:, 0] = W_l.astype(BF16)
        w2[:, 1] = W_r.astype(BF16)

        in_maps.append({"xg": xg, "xo": xo, "w2": w2})
        orders.append(order)

    return in_maps, orders, m_k, b_l


def _assemble(results, orders):
    out = np.empty((N_NODES, 1), dtype=F32)
    for c in range(N_CORES):
        oc = np.asarray(results[c]["out"]).reshape(-1)  # [NODES_PAD]
        out[c * NPC + orders[c], 0] = oc[:NPC]
    return out


# -------------------------------------------------------------- device side
def _build_program(m_k, b_l):
    import concourse.bass as bass
    import concourse.mybir as mybir
    from contextlib import ExitStack

    f32 = mybir.dt.float32
    bf16 = mybir.dt.bfloat16
    plan = _plan(m_k)
    groups = plan["groups"]
    gq_last = plan["gq_last"]
    chunks = plan["chunks"]
    Kmax = len(m_k)
    W_slots = sum(m_k)
    W_acc = NODES_PAD
    offs = np.zeros(Kmax + 1, dtype=np.int64)
    np.cumsum(np.asarray(m_k), out=offs[1:])
    NG = len(groups)

    nc = bass.Bass()
    xg = nc.declare_dram_parameter("xg", [P, W_slots], bf16, isOutput=False)
    xo = nc.declare_dram_parameter("xo", [P, NODES_PAD], bf16, isOutput=False)
    w2_d = nc.declare_dram_parameter("w2", [P, 2], bf16, isOutput=False)
    out = nc.declare_dram_parameter("out", [1, NODES_PAD], f32, isOutput=True)

    with ExitStack() as ctx:
        block = ctx.enter_context(nc.Block())
        s_a0 = ctx.enter_context(nc.semaphore("s_a0"))  # slot0 -> acc landed
        s_w = ctx.enter_context(nc.semaphore("s_w"))  # w2 + xo landed
        s_v = ctx.enter_context(nc.semaphore("s_v"))  # chain folds completed
        s_p = ctx.enter_context(nc.semaphore("s_p"))  # PE chunk pairs done
        s_ad = ctx.enter_context(nc.semaphore("s_ad"))  # ACT chunks drained
        s_out = ctx.enter_context(nc.semaphore("s_out"))
        sg = [ctx.enter_context(nc.semaphore(f"sg{b}")) for b in range(NRING)]

        w_t = ctx.enter_context(nc.sbuf_tensor("w_t", [P, 2], bf16))
        acc = ctx.enter_context(nc.sbuf_tensor("acc", [P, W_acc], bf16))
        xo_t = ctx.enter_context(nc.sbuf_tensor("xo_t", [P, NODES_PAD], bf16))
        orow = ctx.enter_context(nc.sbuf_tensor("orow", [1, NODES_PAD], f32))
        gq = [
            ctx.enter_context(nc.sbuf_tensor(f"gq{b}", [P, GROUP_TARGET], bf16))
            for b in range(NRING)
        ]
        ps = [
            ctx.enter_context(nc.psum_tensor(f"ps{i}", [1, CHUNK], f32))
            for i in range(NPSUM)
        ]

        @block.sync
        def _(sync):
            # slot 0 leads so the DVE chain can start ASAP; w2/xo are only
            # needed by PE mid-stream, so they ride after a few groups
            sync.dma_start(out=acc[:], in_=xg[:, : int(offs[1])]).then_inc(
                s_a0, 16
            )
            for g, (lo, hi) in enumerate(groups):
                b = g % NRING
                if g >= NRING:
                    sync.wait_ge(s_v, gq_last[g - NRING] + 1)
                width = int(offs[hi] - offs[lo])
                sync.dma_start(
                    out=gq[b][:, :width],
                    in_=xg[:, int(offs[lo]) : int(offs[hi])],
                ).then_inc(sg[b], 16)
                if g == min(3, NG - 1):
                    sync.dma_start(out=w_t[:], in_=w2_d[:]).then_inc(s_w, 16)
                    sync.dma_start(out=xo_t[:], in_=xo[:]).then_inc(s_w, 16)
            if NG == 0:
                sync.dma_start(out=w_t[:], in_=w2_d[:]).then_inc(s_w, 16)
                sync.dma_start(out=xo_t[:], in_=xo[:]).then_inc(s_w, 16)
            # all but the highest-degree 128 nodes
            sync.wait_ge(s_ad, len(chunks) - 1)
            sync.dma_start(out=out[:, P:], in_=orow[:, P:]).then_inc(s_out, 16)
            sync.wait_ge(s_ad, len(chunks))
            sync.dma_start(out=out[:, :P], in_=orow[:, :P]).then_inc(s_out, 16)
            sync.wait_ge(s_out, 32)

        @block.vector
        def _(v):
            v.wait_ge(s_a0, 16)
            use = [0] * NRING
            for g, (lo, hi) in enumerate(groups):
                b = g % NRING
                use[b] += 1
                v.wait_ge(sg[b], 16 * use[b])
                for k in range(lo, hi):
                    m = m_k[k]
                    goff = int(offs[k] - offs[lo])
                    v.tensor_tensor(
                        out=acc[:, :m],
                        in0=acc[:, :m],
                        in1=gq[b][:, goff : goff + m],
                        op=mybir.AluOpType.max,
                    ).then_inc(s_v, 1)

        @block.tensor
        def _(te):
            te.wait_ge(s_w, 32)
            for i, (c0, c1, need) in enumerate(chunks):
                wdt = c1 - c0
                if need > 0:
                    te.wait_ge(s_v, need)
                if i >= NPSUM:
                    te.wait_ge(s_ad, i - NPSUM + 1)
                pb = ps[i % NPSUM]
                te.matmul(
                    pb[:, :wdt],
                    w_t[:, 0:1],
                    acc[:, c0:c1],
                    start=True,
                    stop=False,
                )
                te.matmul(
                    pb[:, :wdt],
                    w_t[:, 1:2],
                    xo_t[:, c0:c1],
                    start=False,
                    stop=True,
                ).then_inc(s_p, 1)

        @block.scalar
        def _(a):
            for i, (c0, c1, need) in enumerate(chunks):
                wdt = c1 - c0
                a.wait_ge(s_p, i + 1)
                if b_l == 0.0:
                    ins = a.activation(
                        out=orow[:, c0:c1],
                        in_=ps[i % NPSUM][:, :wdt],
                        func=mybir.ActivationFunctionType.Copy,
                    )
                else:
                    ins = a.activation(
                        out=orow[:, c0:c1],
                        in_=ps[i % NPSUM][:, :wdt],
                        func=mybir.ActivationFunctionType.Identity,
                        bias=float(b_l),
                    )
                ins.then_inc(s_ad, 1)

    return nc


# ---------------------------------------------------------------- entry
def _run(inputs, trace=False, trace_cores=None):
    from concourse.bass_utils import run_bass_kernel_spmd

    in_maps, orders, m_k, b_l = _preprocess(**inputs)
    nc = _build_program(m_k, b_l)
    res = run_bass_kernel_spmd(
        nc,
        in_maps,
        core_ids=list(range(N_CORES)),
        trace=trace,
        trace_cores=trace_cores,
    )
    return _assemble(res.results, orders), res


def kernel(**inputs):
    out, _ = _run(inputs)
    return out
